# revision 44
# baseline (speedup 1.0000x reference)
"""GraphConv (DeepChem) Bass kernel for 8 Trainium2 NeuronCores.

Sharding: data-parallel over rows within each degree bucket. Each core owns
1/8 of every bucket (deg0: 1500 rows, deg1-10: 3750 rows each). W/b
replicated.

Host-side prep is layout + encoding only (gather, transpose, quantize); all
graph-conv arithmetic (neighbor sums, matmuls, bias) runs on device.

Stream encoding: fp8 e3m4 (1B/elem) with sigma-delta error feedback along
each output row's neighbor chain: q_j = rnd(x_j + r_{j-1}), r_j = input -
q_j. The device accumulates W @ q_j over j in PSUM, so the neighbor-sum
error telescopes to a single rounding residual (~0.5% rel) instead of
sqrt(d) independent ones. Self rows are plain e3m4 (~1.3% rel on a 1/(d+1)
signal share). W/bias kept exact-ish in fp16/fp32 (PE supports mixed-dtype
matmul: fp16 lhsT x fp8 rhs). Total rel_fro ~0.8%, budget 2e-2.

Device algorithm per batch of B(<=4) tiles of degree d (degrees issued
descending so the pipeline drains on the smallest batches):
  - ONE contiguous HWDGE load (stream slice [128, (d+1)*B*128] fp8,
    j-major: slot j's B tiles contiguous -> each slot is one N=B*128
    matmul with a shared stationary W)
  - PE: psum[128, B*128] fp32 accumulates sum_j W_nb^T @ Gj^T + W_self^T
    @ selfT (d+1 matmuls, 2 ldweights)
  - DVE/Act eviction adds bias (per-partition scalar) -> fp16 store
Host un-transposes, upcasts, and re-concatenates bucket shards.
"""
import os
import sys
import types
import numpy as np
import ml_dtypes

import concourse.bacc as bacc
import concourse.mybir as mybir
import concourse.tile as tile
from concourse.bass_utils import run_bass_kernel_spmd

N_DEG0 = 12000
N_PER_DEG = 30000
MAX_DEG = 10
D = 128
N_NODES = N_DEG0 + MAX_DEG * N_PER_DEG  # 312000
N_PARAMS = 2 * MAX_DEG + 1  # 21
N_CORES = 8

C_DEG0 = N_DEG0 // N_CORES          # 1500
C_DEG = N_PER_DEG // N_CORES        # 3750
LOCAL_COLS = C_DEG0 + MAX_DEG * C_DEG  # 39000 local rows per core (no pad)

GB = 512                            # cols per PSUM group (bank = 512 fp32)

F8 = mybir.dt.float8e3
F16 = mybir.dt.float16
NP_F8 = ml_dtypes.float8_e3m4
# (DoubleRow fp8e4 was tried: PE busy identical — TRN2 ingests 1 fp8
# row/cycle here, no double-pump — so e3m4's lower quant error wins.)

# degree order: d=1 and deg0 first (small loads land fast; their thin PE
# work covers the cold-DMA ramp while big-degree loads buffer up behind),
# then descending; drain ends on d=2's small final load
DEG_ORDER = [1, 0] + list(range(MAX_DEG, 1, -1))

# cols per DMA load, by degree: DMA issue costs ~0.6us of engine time each
# (DIRECT2D descriptor gen), so loads are as large as SBUF allows. All
# sizes in graph-row columns (the matmul free dim is arbitrary), so the
# 3750-col buckets need no padding to a tile grid.
LOAD_COLS = {0: [1500], 1: [3072, 678], 2: [1536, 1536, 678]}
_DEF_LOAD = [1024, 1024, 1024, 678]
OUT_COLS = 2048                     # cols per output store chunk


def _plan(ncols, step):
    plan = []
    left = ncols
    while left > 0:
        b = min(step, left)
        plan.append(b)
        left -= b
    return plan


def _cols(d):
    return C_DEG if d > 0 else C_DEG0


# load schedule: (d, col0, Lc, strm_col_base)
LOADS = []
_soff = 0
_ooff = 0
OUT_BASE = {}
LOAD_SOFF = {}
for _d in DEG_ORDER:
    OUT_BASE[_d] = _ooff
    _nblk = _d + 1 if _d > 0 else 1
    _t0 = 0
    for _L in LOAD_COLS.get(_d, _DEF_LOAD):
        LOADS.append((_d, _t0, _L, _soff))
        LOAD_SOFF[(_d, _t0)] = (_soff, _L)
        _soff += _nblk * _L
        _t0 += _L
    _ooff += _cols(_d)
SCOLS = _soff                       # 245250
assert _ooff == LOCAL_COLS

_COMPILED = None
LAST_RESULT = None


def _maybe_install_trace_hook():
    """Inject antenv.axon_hooks so trace=True can NTFF-profile under axon."""
    try:
        import antenv.axon_hooks  # noqa: F401
        return True
    except ImportError:
        pass
    try:
        hooks = types.ModuleType("antenv.axon_hooks")
        hooks._hook = None

        def _set(h):
            hooks._hook = h

        def _get():
            return hooks._hook

        hooks.set_axon_ntff_profile_hook = _set
        hooks.get_axon_ntff_profile_hook = _get
        sys.modules["antenv.axon_hooks"] = hooks
        import antenv

        antenv.axon_hooks = hooks
        from trn_agent_boot.trn_boot import _ntff_profile_via_ctypes

        _set(_ntff_profile_via_ctypes("/opt/axon/libaxon_pjrt.so"))
        return True
    except Exception:
        return False


def _build():
    nc = bacc.Bacc()
    strm = nc.declare_dram_parameter("strm", [D, SCOLS], F8, isOutput=False)
    w_in = nc.declare_dram_parameter(
        "w", [128, N_PARAMS * 128], F16, isOutput=False
    )
    bsumT = nc.declare_dram_parameter(
        "bsumT", [D, MAX_DEG + 1], mybir.dt.float32, isOutput=False
    )
    outT = nc.declare_dram_parameter(
        "outT", [D, LOCAL_COLS], F16, isOutput=True
    )

    with tile.TileContext(nc) as tc:
        with (
            tc.tile_pool(name="const", bufs=1) as constp,
            tc.tile_pool(name="gp", bufs=14) as gp,
            tc.tile_pool(name="obp", bufs=6) as obp,
            tc.tile_pool(name="psout", bufs=8, space="PSUM") as psout,
        ):
            # W[0] split out so the first (deg0) matmuls only wait on a
            # 32KB load; the bulk of W streams in parallel on another queue
            w0_sb = constp.tile([128, 128], F16)
            nc.scalar.dma_start(out=w0_sb[:], in_=w_in[:, 0:128])
            bs_sb = constp.tile([128, MAX_DEG + 1], mybir.dt.float32)
            nc.scalar.dma_start(out=bs_sb[:], in_=bsumT[:, :])
            w_sb = constp.tile([128, N_PARAMS * 128], F16)
            nc.scalar.dma_start(out=w_sb[:], in_=w_in[:, :])

            gi = 0   # global PSUM-group counter (engine alternation)
            for d in DEG_ORDER:
                ncd = _cols(d)
                nblk = d + 1 if d > 0 else 1
                obase = OUT_BASE[d]
                g = None
                lt0 = lL = 0
                ob = None
                ot0 = oL = 0
                gt0 = 0
                for R in _plan(ncd, GB):
                    if g is None or gt0 >= lt0 + lL:
                        lt0 = gt0
                        soff, lL = LOAD_SOFF[(d, lt0)]
                        ncols = nblk * lL
                        g = gp.tile([128, nblk, lL], F8, tag="g")
                        nc.sync.dma_start(
                            out=g[:], in_=strm[:, soff:soff + ncols]
                        )
                    if ob is None or gt0 >= ot0 + oL:
                        ot0 = gt0
                        oL = min(OUT_COLS, ncd - ot0)
                        ob = obp.tile([128, oL], F16, tag="ob")
                    c0 = gt0 - lt0
                    ps = psout.tile([128, R], mybir.dt.float32, tag="ps")
                    if d > 0:
                        for j in range(d):
                            nc.tensor.matmul(
                                out=ps[:],
                                lhsT=w_sb[:, (2 * d - 1) * 128:
                                          (2 * d) * 128],
                                rhs=g[:, j, c0:c0 + R],
                                start=(j == 0),
                                stop=False,
                            )
                        nc.tensor.matmul(
                            out=ps[:],
                            lhsT=w_sb[:, (2 * d) * 128:(2 * d + 1) * 128],
                            rhs=g[:, d, c0:c0 + R],
                            start=False,
                            stop=True,
                        )
                    else:
                        nc.tensor.matmul(
                            out=ps[:],
                            lhsT=w0_sb[:],
                            rhs=g[:, 0, c0:c0 + R],
                            start=True,
                            stop=True,
                        )
                    oc0 = gt0 - ot0
                    if gi % 2 == 0:
                        nc.vector.tensor_scalar_add(
                            out=ob[:, oc0:oc0 + R],
                            in0=ps[:],
                            scalar1=bs_sb[:, d:d + 1],
                        )
                    else:
                        nc.scalar.add(
                            out=ob[:, oc0:oc0 + R],
                            in_=ps[:],
                            add=bs_sb[:, d:d + 1],
                        )
                    gi += 1
                    gt0 += R
                    if gt0 >= ot0 + oL:
                        nc.scalar.dma_start(
                            out=outT[:, obase + ot0:obase + ot0 + oL],
                            in_=ob[:],
                        )

    nc.compile()
    return nc


def _quantize_feedback(nf32, adj):
    """Sigma-delta e3m4 quantization of gathered neighbors along axis 1."""
    n, dd = adj.shape
    Q = np.empty((n, dd, D), NP_F8)
    r = np.zeros((n, D), np.float32)
    for j in range(dd):
        t = nf32[adj[:, j]]
        t += r
        q = t.astype(NP_F8)
        Q[:, j] = q
        r = t - q.astype(np.float32)
    return Q


def kernel(node_features, deg_slice, adj1, adj2, adj3, adj4, adj5, adj6,
           adj7, adj8, adj9, adj10, W, b):
    global _COMPILED, LAST_RESULT
    nf32 = np.ascontiguousarray(np.asarray(node_features, dtype=np.float32))
    adjs = [np.asarray(a, dtype=np.int32)
            for a in (adj1, adj2, adj3, adj4, adj5, adj6, adj7, adj8, adj9,
                      adj10)]
    Wf = np.asarray(W, dtype=np.float32)
    bf = np.asarray(b, dtype=np.float32)

    # weights packed [din, k*128+dout] fp16
    WT32 = np.ascontiguousarray(Wf.transpose(1, 0, 2))  # [din, k, dout]
    wpack = WT32.reshape(D, N_PARAMS * D).astype(np.float16)

    # bias pre-sum (affine marshalling): bsum[0]=b[0]; bsum[d]=b[2d-1]+b[2d]
    bsum = np.empty((MAX_DEG + 1, D), np.float32)
    bsum[0] = bf[0]
    for d in range(1, MAX_DEG + 1):
        bsum[d] = bf[2 * d - 1] + bf[2 * d]
    bsumT = np.ascontiguousarray(bsum.T)

    # quantize once (shared across cores), then carve per-core streams
    Qn = {d: _quantize_feedback(nf32, adjs[d - 1])
          for d in range(1, MAX_DEG + 1)}
    Sq = {d: nf32[N_DEG0 + (d - 1) * N_PER_DEG:
                  N_DEG0 + d * N_PER_DEG].astype(NP_F8)
          for d in range(1, MAX_DEG + 1)}
    Sq[0] = nf32[0:N_DEG0].astype(NP_F8)

    in_maps = []
    for c in range(N_CORES):
        strm = np.empty((D, SCOLS), NP_F8)
        # per-degree [rows, slots, din] arrays for this core (no padding)
        deg_arr = {}
        for d in DEG_ORDER:
            if d > 0:
                a = np.empty((C_DEG, d + 1, D), NP_F8)
                a[:, :d] = Qn[d][c * C_DEG:(c + 1) * C_DEG]
                a[:, d] = Sq[d][c * C_DEG:(c + 1) * C_DEG]
            else:
                a = Sq[0][c * C_DEG0:(c + 1) * C_DEG0].reshape(
                    C_DEG0, 1, D
                )
            deg_arr[d] = a
        for (d, t0, L, soff) in LOADS:
            nblk = d + 1 if d > 0 else 1
            ncols = nblk * L
            chunk = deg_arr[d][t0:t0 + L]  # [L, nblk, D]
            strm[:, soff:soff + ncols] = (
                chunk.transpose(2, 1, 0).reshape(D, ncols)
            )
        in_maps.append({
            "strm": strm,
            "w": wpack,
            "bsumT": bsumT,
        })

    if _COMPILED is None:
        _COMPILED = _build()

    trace = bool(int(os.environ.get("KERNEL_TRACE", "0")))
    if trace:
        trace = _maybe_install_trace_hook()
    res = run_bass_kernel_spmd(
        _COMPILED, in_maps, core_ids=list(range(N_CORES)), trace=trace
    )
    LAST_RESULT = res

    out = np.empty((N_NODES, D), np.float32)
    for c in range(N_CORES):
        oT = res.results[c]["outT"].astype(np.float32)
        for d in DEG_ORDER:
            base = OUT_BASE[d]
            if d > 0:
                gs = N_DEG0 + (d - 1) * N_PER_DEG + c * C_DEG
                out[gs:gs + C_DEG] = oT[:, base:base + C_DEG].T
            else:
                out[c * C_DEG0:(c + 1) * C_DEG0] = (
                    oT[:, base:base + C_DEG0].T
                )
    return out


# revision 45
# speedup vs baseline: 1.0337x; 1.0337x over previous
"""GraphConv (DeepChem) Bass kernel for 8 Trainium2 NeuronCores.

Sharding: data-parallel over rows within each degree bucket. Each core owns
1/8 of every bucket (deg0: 1500 rows, deg1-10: 3750 rows each). W/b
replicated.

Host-side prep is layout + encoding only (gather, transpose, quantize); all
graph-conv arithmetic (neighbor sums, matmuls, bias) runs on device.

Stream encoding: fp8 e3m4 (1B/elem) with sigma-delta error feedback along
each output row's neighbor chain: q_j = rnd(x_j + r_{j-1}). The device
accumulates W @ q_j over j in fp32 PSUM, so the neighbor-sum error
telescopes to a single rounding residual (~0.5% rel) instead of sqrt(d)
independent ones. Self rows are plain e3m4 (~1.3% rel on a 1/(d+1) signal
share). W stays fp16 (the PE supports mixed-dtype matmul: fp16 lhsT x fp8
rhs at full rate). Measured rel_fro 7.5e-3 vs the 2e-2 budget.

Schedule (all sizes in graph-row columns; the matmul free dim is
arbitrary, so buckets need no padding to a 128 grid):
  - degrees ordered [1, 0, 10, 9, ..., 2]: the small-load, thin-compute
    buckets cover the cold-DMA ramp, and the run drains on d=2's small
    final load;
  - per degree, big HWDGE loads (~1-11KB/partition; each dma_start costs
    ~0.6us of issuing-engine time, so loads are few and large), slot-major
    [128, d+1, Lc] fp8 tiles, 12-deep ring for prefetch;
  - per 512-col PSUM group: d+1 matmuls (shared stationary W per slot
    set, N=512 free dim) accumulate sum_j W_nb^T @ Gj^T + W_self^T @
    selfT in one PSUM bank; start/stop flags delimit the accumulation;
  - eviction adds the pre-summed bias (per-partition scalar) and casts to
    fp16 on DVE/Act (alternating engines), 2048-col output chunks stored
    via scalar-queue DMAs.
Host un-transposes, upcasts, and re-concatenates bucket shards.

Perf notes (measured): PE is the bottleneck at ~109us busy (245k output
cols x 1 cycle/col at 2.4GHz; fp8 DoubleRow gave no speedup on this HW),
total DMA ~42MB/core at ~400GB/s effective; HW exec ~133-138us, vs 235us
for the bf16 host-gather baseline.
"""
import os
import sys
import types
import numpy as np
import ml_dtypes

import concourse.bacc as bacc
import concourse.mybir as mybir
import concourse.tile as tile
from concourse.bass_utils import run_bass_kernel_spmd

N_DEG0 = 12000
N_PER_DEG = 30000
MAX_DEG = 10
D = 128
N_NODES = N_DEG0 + MAX_DEG * N_PER_DEG  # 312000
N_PARAMS = 2 * MAX_DEG + 1  # 21
N_CORES = 8

C_DEG0 = N_DEG0 // N_CORES          # 1500
C_DEG = N_PER_DEG // N_CORES        # 3750
LOCAL_COLS = C_DEG0 + MAX_DEG * C_DEG  # 39000 local rows per core (no pad)

GB = 512                            # cols per PSUM group (bank = 512 fp32)

F8 = mybir.dt.float8e3
F16 = mybir.dt.float16
NP_F8 = ml_dtypes.float8_e3m4
# (DoubleRow fp8e4 was tried: PE busy identical — TRN2 ingests 1 fp8
# row/cycle here, no double-pump — so e3m4's lower quant error wins.)

# degree order: d=1 and deg0 first (small loads land fast; their thin PE
# work covers the cold-DMA ramp while big-degree loads buffer up behind),
# then descending; drain ends on d=2's small final load
DEG_ORDER = [1, 0] + list(range(MAX_DEG, 1, -1))

# cols per DMA load, by degree: DMA issue costs ~0.6us of engine time each
# (DIRECT2D descriptor gen), so loads are as large as SBUF allows. All
# sizes in graph-row columns (the matmul free dim is arbitrary), so the
# 3750-col buckets need no padding to a tile grid.
LOAD_COLS = {0: [1500], 1: [3072, 678], 2: [1536, 1536, 678]}
_DEF_LOAD = [1024, 1024, 1024, 678]
OUT_COLS = 2048                     # cols per output store chunk


def _plan(ncols, step):
    plan = []
    left = ncols
    while left > 0:
        b = min(step, left)
        plan.append(b)
        left -= b
    return plan


def _cols(d):
    return C_DEG if d > 0 else C_DEG0


# load schedule: (d, col0, Lc, strm_col_base)
LOADS = []
_soff = 0
_ooff = 0
OUT_BASE = {}
LOAD_SOFF = {}
for _d in DEG_ORDER:
    OUT_BASE[_d] = _ooff
    _nblk = _d + 1 if _d > 0 else 1
    _t0 = 0
    for _L in LOAD_COLS.get(_d, _DEF_LOAD):
        LOADS.append((_d, _t0, _L, _soff))
        LOAD_SOFF[(_d, _t0)] = (_soff, _L)
        _soff += _nblk * _L
        _t0 += _L
    _ooff += _cols(_d)
SCOLS = _soff                       # 245250
assert _ooff == LOCAL_COLS

_COMPILED = None
LAST_RESULT = None


def _maybe_install_trace_hook():
    """Inject antenv.axon_hooks so trace=True can NTFF-profile under axon."""
    try:
        import antenv.axon_hooks  # noqa: F401
        return True
    except ImportError:
        pass
    try:
        hooks = types.ModuleType("antenv.axon_hooks")
        hooks._hook = None

        def _set(h):
            hooks._hook = h

        def _get():
            return hooks._hook

        hooks.set_axon_ntff_profile_hook = _set
        hooks.get_axon_ntff_profile_hook = _get
        sys.modules["antenv.axon_hooks"] = hooks
        import antenv

        antenv.axon_hooks = hooks
        from trn_agent_boot.trn_boot import _ntff_profile_via_ctypes

        _set(_ntff_profile_via_ctypes("/opt/axon/libaxon_pjrt.so"))
        return True
    except Exception:
        return False


def _build():
    nc = bacc.Bacc()
    strm = nc.declare_dram_parameter("strm", [D, SCOLS], F8, isOutput=False)
    w_in = nc.declare_dram_parameter(
        "w", [128, N_PARAMS * 128], F16, isOutput=False
    )
    bsumT = nc.declare_dram_parameter(
        "bsumT", [D, MAX_DEG + 1], mybir.dt.float32, isOutput=False
    )
    outT = nc.declare_dram_parameter(
        "outT", [D, LOCAL_COLS], F16, isOutput=True
    )

    with tile.TileContext(nc) as tc:
        with (
            tc.tile_pool(name="const", bufs=1) as constp,
            tc.tile_pool(name="gp", bufs=12) as gp,
            tc.tile_pool(name="obp", bufs=6) as obp,
            tc.tile_pool(name="psout", bufs=8, space="PSUM") as psout,
        ):
            # W[0] split out so the first (deg0) matmuls only wait on a
            # 32KB load; the bulk of W streams in parallel on another queue
            w0_sb = constp.tile([128, 128], F16)
            nc.scalar.dma_start(out=w0_sb[:], in_=w_in[:, 0:128])
            bs_sb = constp.tile([128, MAX_DEG + 1], mybir.dt.float32)
            nc.scalar.dma_start(out=bs_sb[:], in_=bsumT[:, :])
            w_sb = constp.tile([128, N_PARAMS * 128], F16)
            nc.scalar.dma_start(out=w_sb[:], in_=w_in[:, :])

            gi = 0   # global PSUM-group counter (engine alternation)
            for d in DEG_ORDER:
                ncd = _cols(d)
                nblk = d + 1 if d > 0 else 1
                obase = OUT_BASE[d]
                g = None
                lt0 = lL = 0
                ob = None
                ot0 = oL = 0
                gt0 = 0
                for R in _plan(ncd, GB):
                    if g is None or gt0 >= lt0 + lL:
                        lt0 = gt0
                        soff, lL = LOAD_SOFF[(d, lt0)]
                        ncols = nblk * lL
                        g = gp.tile([128, nblk, lL], F8, tag="g")
                        nc.sync.dma_start(
                            out=g[:], in_=strm[:, soff:soff + ncols]
                        )
                    if ob is None or gt0 >= ot0 + oL:
                        ot0 = gt0
                        oL = min(OUT_COLS, ncd - ot0)
                        ob = obp.tile([128, oL], F16, tag="ob")
                    c0 = gt0 - lt0
                    ps = psout.tile([128, R], mybir.dt.float32, tag="ps")
                    if d > 0:
                        for j in range(d):
                            nc.tensor.matmul(
                                out=ps[:],
                                lhsT=w_sb[:, (2 * d - 1) * 128:
                                          (2 * d) * 128],
                                rhs=g[:, j, c0:c0 + R],
                                start=(j == 0),
                                stop=False,
                            )
                        nc.tensor.matmul(
                            out=ps[:],
                            lhsT=w_sb[:, (2 * d) * 128:(2 * d + 1) * 128],
                            rhs=g[:, d, c0:c0 + R],
                            start=False,
                            stop=True,
                        )
                    else:
                        nc.tensor.matmul(
                            out=ps[:],
                            lhsT=w0_sb[:],
                            rhs=g[:, 0, c0:c0 + R],
                            start=True,
                            stop=True,
                        )
                    oc0 = gt0 - ot0
                    if gi % 2 == 0:
                        nc.vector.tensor_scalar_add(
                            out=ob[:, oc0:oc0 + R],
                            in0=ps[:],
                            scalar1=bs_sb[:, d:d + 1],
                        )
                    else:
                        nc.scalar.add(
                            out=ob[:, oc0:oc0 + R],
                            in_=ps[:],
                            add=bs_sb[:, d:d + 1],
                        )
                    gi += 1
                    gt0 += R
                    if gt0 >= ot0 + oL:
                        nc.scalar.dma_start(
                            out=outT[:, obase + ot0:obase + ot0 + oL],
                            in_=ob[:],
                        )

    nc.compile()
    return nc


def _quantize_feedback(nf32, adj):
    """Sigma-delta e3m4 quantization of gathered neighbors along axis 1."""
    n, dd = adj.shape
    Q = np.empty((n, dd, D), NP_F8)
    r = np.zeros((n, D), np.float32)
    for j in range(dd):
        t = nf32[adj[:, j]]
        t += r
        q = t.astype(NP_F8)
        Q[:, j] = q
        r = t - q.astype(np.float32)
    return Q


def kernel(node_features, deg_slice, adj1, adj2, adj3, adj4, adj5, adj6,
           adj7, adj8, adj9, adj10, W, b):
    global _COMPILED, LAST_RESULT
    nf32 = np.ascontiguousarray(np.asarray(node_features, dtype=np.float32))
    adjs = [np.asarray(a, dtype=np.int32)
            for a in (adj1, adj2, adj3, adj4, adj5, adj6, adj7, adj8, adj9,
                      adj10)]
    Wf = np.asarray(W, dtype=np.float32)
    bf = np.asarray(b, dtype=np.float32)

    # weights packed [din, k*128+dout] fp16
    WT32 = np.ascontiguousarray(Wf.transpose(1, 0, 2))  # [din, k, dout]
    wpack = WT32.reshape(D, N_PARAMS * D).astype(np.float16)

    # bias pre-sum (affine marshalling): bsum[0]=b[0]; bsum[d]=b[2d-1]+b[2d]
    bsum = np.empty((MAX_DEG + 1, D), np.float32)
    bsum[0] = bf[0]
    for d in range(1, MAX_DEG + 1):
        bsum[d] = bf[2 * d - 1] + bf[2 * d]
    bsumT = np.ascontiguousarray(bsum.T)

    # quantize once (shared across cores), then carve per-core streams
    Qn = {d: _quantize_feedback(nf32, adjs[d - 1])
          for d in range(1, MAX_DEG + 1)}
    Sq = {d: nf32[N_DEG0 + (d - 1) * N_PER_DEG:
                  N_DEG0 + d * N_PER_DEG].astype(NP_F8)
          for d in range(1, MAX_DEG + 1)}
    Sq[0] = nf32[0:N_DEG0].astype(NP_F8)

    in_maps = []
    for c in range(N_CORES):
        strm = np.empty((D, SCOLS), NP_F8)
        # per-degree [rows, slots, din] arrays for this core (no padding)
        deg_arr = {}
        for d in DEG_ORDER:
            if d > 0:
                a = np.empty((C_DEG, d + 1, D), NP_F8)
                a[:, :d] = Qn[d][c * C_DEG:(c + 1) * C_DEG]
                a[:, d] = Sq[d][c * C_DEG:(c + 1) * C_DEG]
            else:
                a = Sq[0][c * C_DEG0:(c + 1) * C_DEG0].reshape(
                    C_DEG0, 1, D
                )
            deg_arr[d] = a
        for (d, t0, L, soff) in LOADS:
            nblk = d + 1 if d > 0 else 1
            ncols = nblk * L
            chunk = deg_arr[d][t0:t0 + L]  # [L, nblk, D]
            strm[:, soff:soff + ncols] = (
                chunk.transpose(2, 1, 0).reshape(D, ncols)
            )
        in_maps.append({
            "strm": strm,
            "w": wpack,
            "bsumT": bsumT,
        })

    if _COMPILED is None:
        _COMPILED = _build()

    trace = bool(int(os.environ.get("KERNEL_TRACE", "0")))
    if trace:
        trace = _maybe_install_trace_hook()
    res = run_bass_kernel_spmd(
        _COMPILED, in_maps, core_ids=list(range(N_CORES)), trace=trace
    )
    LAST_RESULT = res

    out = np.empty((N_NODES, D), np.float32)
    for c in range(N_CORES):
        oT = res.results[c]["outT"].astype(np.float32)
        for d in DEG_ORDER:
            base = OUT_BASE[d]
            if d > 0:
                gs = N_DEG0 + (d - 1) * N_PER_DEG + c * C_DEG
                out[gs:gs + C_DEG] = oT[:, base:base + C_DEG].T
            else:
                out[c * C_DEG0:(c + 1) * C_DEG0] = (
                    oT[:, base:base + C_DEG0].T
                )
    return out


# revision 46
# speedup vs baseline: 1.0352x; 1.0015x over previous
"""GraphConv (DeepChem) Bass kernel for 8 Trainium2 NeuronCores.

Sharding: data-parallel over rows within each degree bucket. Each core owns
1/8 of every bucket (deg0: 1500 rows, deg1-10: 3750 rows each). W/b
replicated.

Host-side prep is layout + encoding only (gather, transpose, quantize); all
graph-conv arithmetic (neighbor sums, matmuls, bias) runs on device.

Stream encoding: fp8 e3m4 (1B/elem) with sigma-delta error feedback along
each output row's neighbor chain: q_j = rnd(x_j + r_{j-1}), r_j = input -
q_j. The device accumulates W @ q_j over j in PSUM, so the neighbor-sum
error telescopes to a single rounding residual (~0.5% rel) instead of
sqrt(d) independent ones. Self rows are plain e3m4 (~1.3% rel on a 1/(d+1)
signal share). W/bias kept exact-ish in fp16/fp32 (PE supports mixed-dtype
matmul: fp16 lhsT x fp8 rhs). Total rel_fro ~0.8%, budget 2e-2.

Device algorithm per batch of B(<=4) tiles of degree d (degrees issued
descending so the pipeline drains on the smallest batches):
  - ONE contiguous HWDGE load (stream slice [128, (d+1)*B*128] fp8,
    j-major: slot j's B tiles contiguous -> each slot is one N=B*128
    matmul with a shared stationary W)
  - PE: psum[128, B*128] fp32 accumulates sum_j W_nb^T @ Gj^T + W_self^T
    @ selfT (d+1 matmuls, 2 ldweights)
  - DVE/Act eviction adds bias (per-partition scalar) -> fp16 store
Host un-transposes, upcasts, and re-concatenates bucket shards.
"""
import os
import sys
import types
import numpy as np
import ml_dtypes

import concourse.bacc as bacc
import concourse.mybir as mybir
import concourse.tile as tile
from concourse.bass_utils import run_bass_kernel_spmd

N_DEG0 = 12000
N_PER_DEG = 30000
MAX_DEG = 10
D = 128
N_NODES = N_DEG0 + MAX_DEG * N_PER_DEG  # 312000
N_PARAMS = 2 * MAX_DEG + 1  # 21
N_CORES = 8

C_DEG0 = N_DEG0 // N_CORES          # 1500
C_DEG = N_PER_DEG // N_CORES        # 3750
LOCAL_COLS = C_DEG0 + MAX_DEG * C_DEG  # 39000 local rows per core (no pad)

GB = 512                            # cols per PSUM group (bank = 512 fp32)

F8 = mybir.dt.float8e3
F16 = mybir.dt.float16
NP_F8 = ml_dtypes.float8_e3m4
# (DoubleRow fp8e4 was tried: PE busy identical — TRN2 ingests 1 fp8
# row/cycle here, no double-pump — so e3m4's lower quant error wins.)

# degree order: d=1 and deg0 first (small loads land fast; their thin PE
# work covers the cold-DMA ramp while big-degree loads buffer up behind),
# then descending; drain ends on d=2's small final load
DEG_ORDER = [1, 0] + list(range(MAX_DEG, 1, -1))

# cols per DMA load, by degree: DMA issue costs ~0.6us of engine time each
# (DIRECT2D descriptor gen), so loads are as large as SBUF allows. All
# sizes in graph-row columns (the matmul free dim is arbitrary), so the
# 3750-col buckets need no padding to a tile grid.
LOAD_COLS = {0: [1500], 1: [3072, 678], 2: [1536, 1536, 678]}
_DEF_LOAD = [1024, 1024, 1024, 678]
OUT_COLS = 2048                     # cols per output store chunk


def _plan(ncols, step):
    plan = []
    left = ncols
    while left > 0:
        b = min(step, left)
        plan.append(b)
        left -= b
    return plan


def _cols(d):
    return C_DEG if d > 0 else C_DEG0


# load schedule: (d, col0, Lc, strm_col_base)
LOADS = []
_soff = 0
_ooff = 0
OUT_BASE = {}
LOAD_SOFF = {}
for _d in DEG_ORDER:
    OUT_BASE[_d] = _ooff
    _nblk = _d + 1 if _d > 0 else 1
    _t0 = 0
    for _L in LOAD_COLS.get(_d, _DEF_LOAD):
        LOADS.append((_d, _t0, _L, _soff))
        LOAD_SOFF[(_d, _t0)] = (_soff, _L)
        _soff += _nblk * _L
        _t0 += _L
    _ooff += _cols(_d)
SCOLS = _soff                       # 245250
assert _ooff == LOCAL_COLS

_COMPILED = None
LAST_RESULT = None


def _maybe_install_trace_hook():
    """Inject antenv.axon_hooks so trace=True can NTFF-profile under axon."""
    try:
        import antenv.axon_hooks  # noqa: F401
        return True
    except ImportError:
        pass
    try:
        hooks = types.ModuleType("antenv.axon_hooks")
        hooks._hook = None

        def _set(h):
            hooks._hook = h

        def _get():
            return hooks._hook

        hooks.set_axon_ntff_profile_hook = _set
        hooks.get_axon_ntff_profile_hook = _get
        sys.modules["antenv.axon_hooks"] = hooks
        import antenv

        antenv.axon_hooks = hooks
        from trn_agent_boot.trn_boot import _ntff_profile_via_ctypes

        _set(_ntff_profile_via_ctypes("/opt/axon/libaxon_pjrt.so"))
        return True
    except Exception:
        return False


def _build():
    nc = bacc.Bacc()
    strm = nc.declare_dram_parameter("strm", [D, SCOLS], F8, isOutput=False)
    w_in = nc.declare_dram_parameter(
        "w", [128, N_PARAMS * 128], F16, isOutput=False
    )
    bsumT = nc.declare_dram_parameter(
        "bsumT", [D, MAX_DEG + 1], mybir.dt.float32, isOutput=False
    )
    outT = nc.declare_dram_parameter(
        "outT", [D, LOCAL_COLS], F16, isOutput=True
    )

    with tile.TileContext(nc) as tc:
        with (
            tc.tile_pool(name="const", bufs=1) as constp,
            tc.tile_pool(name="gp", bufs=12) as gp,
            tc.tile_pool(name="obp", bufs=6) as obp,
            tc.tile_pool(name="psout", bufs=8, space="PSUM") as psout,
        ):
            # W[0] split out so the first (deg0) matmuls only wait on a
            # 32KB load; the bulk of W streams in parallel on another queue
            w0_sb = constp.tile([128, 128], F16)
            nc.scalar.dma_start(out=w0_sb[:], in_=w_in[:, 0:128])
            bs_sb = constp.tile([128, MAX_DEG + 1], mybir.dt.float32)
            nc.scalar.dma_start(out=bs_sb[:], in_=bsumT[:, :])
            w_sb = constp.tile([128, N_PARAMS * 128], F16)
            nc.scalar.dma_start(out=w_sb[:], in_=w_in[:, :])

            gi = 0   # global PSUM-group counter (engine alternation)
            for d in DEG_ORDER:
                ncd = _cols(d)
                nblk = d + 1 if d > 0 else 1
                obase = OUT_BASE[d]
                g = None
                lt0 = lL = 0
                ob = None
                ot0 = oL = 0
                gt0 = 0
                for R in _plan(ncd, GB):
                    if g is None or gt0 >= lt0 + lL:
                        lt0 = gt0
                        soff, lL = LOAD_SOFF[(d, lt0)]
                        ncols = nblk * lL
                        g = gp.tile([128, nblk, lL], F8, tag="g")
                        nc.sync.dma_start(
                            out=g[:], in_=strm[:, soff:soff + ncols]
                        )
                    if ob is None or gt0 >= ot0 + oL:
                        ot0 = gt0
                        oL = min(OUT_COLS, ncd - ot0)
                        ob = obp.tile([128, oL], F16, tag="ob")
                    c0 = gt0 - lt0
                    ps = psout.tile([128, R], mybir.dt.float32, tag="ps")
                    if d > 0:
                        for j in range(d):
                            nc.tensor.matmul(
                                out=ps[:],
                                lhsT=w_sb[:, (2 * d - 1) * 128:
                                          (2 * d) * 128],
                                rhs=g[:, j, c0:c0 + R],
                                start=(j == 0),
                                stop=False,
                            )
                        nc.tensor.matmul(
                            out=ps[:],
                            lhsT=w_sb[:, (2 * d) * 128:(2 * d + 1) * 128],
                            rhs=g[:, d, c0:c0 + R],
                            start=False,
                            stop=True,
                        )
                    else:
                        nc.tensor.matmul(
                            out=ps[:],
                            lhsT=w0_sb[:],
                            rhs=g[:, 0, c0:c0 + R],
                            start=True,
                            stop=True,
                        )
                    oc0 = gt0 - ot0
                    if gi % 2 == 0:
                        nc.vector.tensor_scalar_add(
                            out=ob[:, oc0:oc0 + R],
                            in0=ps[:],
                            scalar1=bs_sb[:, d:d + 1],
                        )
                    else:
                        nc.scalar.add(
                            out=ob[:, oc0:oc0 + R],
                            in_=ps[:],
                            add=bs_sb[:, d:d + 1],
                        )
                    gi += 1
                    gt0 += R
                    if gt0 >= ot0 + oL:
                        nc.scalar.dma_start(
                            out=outT[:, obase + ot0:obase + ot0 + oL],
                            in_=ob[:],
                        )

    nc.compile()
    return nc


def _quantize_feedback(nf32, adj):
    """Sigma-delta e3m4 quantization of gathered neighbors along axis 1."""
    n, dd = adj.shape
    Q = np.empty((n, dd, D), NP_F8)
    r = np.zeros((n, D), np.float32)
    for j in range(dd):
        t = nf32[adj[:, j]]
        t += r
        q = t.astype(NP_F8)
        Q[:, j] = q
        r = t - q.astype(np.float32)
    return Q


def kernel(node_features, deg_slice, adj1, adj2, adj3, adj4, adj5, adj6,
           adj7, adj8, adj9, adj10, W, b):
    global _COMPILED, LAST_RESULT
    nf32 = np.ascontiguousarray(np.asarray(node_features, dtype=np.float32))
    adjs = [np.asarray(a, dtype=np.int32)
            for a in (adj1, adj2, adj3, adj4, adj5, adj6, adj7, adj8, adj9,
                      adj10)]
    Wf = np.asarray(W, dtype=np.float32)
    bf = np.asarray(b, dtype=np.float32)

    # weights packed [din, k*128+dout] fp16
    WT32 = np.ascontiguousarray(Wf.transpose(1, 0, 2))  # [din, k, dout]
    wpack = WT32.reshape(D, N_PARAMS * D).astype(np.float16)

    # bias pre-sum (affine marshalling): bsum[0]=b[0]; bsum[d]=b[2d-1]+b[2d]
    bsum = np.empty((MAX_DEG + 1, D), np.float32)
    bsum[0] = bf[0]
    for d in range(1, MAX_DEG + 1):
        bsum[d] = bf[2 * d - 1] + bf[2 * d]
    bsumT = np.ascontiguousarray(bsum.T)

    # quantize once (shared across cores), then carve per-core streams
    Qn = {d: _quantize_feedback(nf32, adjs[d - 1])
          for d in range(1, MAX_DEG + 1)}
    Sq = {d: nf32[N_DEG0 + (d - 1) * N_PER_DEG:
                  N_DEG0 + d * N_PER_DEG].astype(NP_F8)
          for d in range(1, MAX_DEG + 1)}
    Sq[0] = nf32[0:N_DEG0].astype(NP_F8)

    in_maps = []
    for c in range(N_CORES):
        strm = np.empty((D, SCOLS), NP_F8)
        # per-degree [rows, slots, din] arrays for this core (no padding)
        deg_arr = {}
        for d in DEG_ORDER:
            if d > 0:
                a = np.empty((C_DEG, d + 1, D), NP_F8)
                a[:, :d] = Qn[d][c * C_DEG:(c + 1) * C_DEG]
                a[:, d] = Sq[d][c * C_DEG:(c + 1) * C_DEG]
            else:
                a = Sq[0][c * C_DEG0:(c + 1) * C_DEG0].reshape(
                    C_DEG0, 1, D
                )
            deg_arr[d] = a
        for (d, t0, L, soff) in LOADS:
            nblk = d + 1 if d > 0 else 1
            ncols = nblk * L
            chunk = deg_arr[d][t0:t0 + L]  # [L, nblk, D]
            strm[:, soff:soff + ncols] = (
                chunk.transpose(2, 1, 0).reshape(D, ncols)
            )
        in_maps.append({
            "strm": strm,
            "w": wpack,
            "bsumT": bsumT,
        })

    if _COMPILED is None:
        _COMPILED = _build()

    trace = bool(int(os.environ.get("KERNEL_TRACE", "0")))
    if trace:
        trace = _maybe_install_trace_hook()
    res = run_bass_kernel_spmd(
        _COMPILED, in_maps, core_ids=list(range(N_CORES)), trace=trace
    )
    LAST_RESULT = res

    out = np.empty((N_NODES, D), np.float32)
    for c in range(N_CORES):
        oT = res.results[c]["outT"].astype(np.float32)
        for d in DEG_ORDER:
            base = OUT_BASE[d]
            if d > 0:
                gs = N_DEG0 + (d - 1) * N_PER_DEG + c * C_DEG
                out[gs:gs + C_DEG] = oT[:, base:base + C_DEG].T
            else:
                out[c * C_DEG0:(c + 1) * C_DEG0] = (
                    oT[:, base:base + C_DEG0].T
                )
    return out
